# revision 2
# baseline (speedup 1.0000x reference)
"""GQA causal prefill attention on 8 TRN2 NeuronCores.

Sharding: head-parallel. Core c computes q heads [4c, 4c+4) against kv head c
(n_rep = 4, so the GQA groups align exactly with the shard; no cross-core
communication).

Per-core algorithm (T=2048 tokens, 4 q heads, head_dim 128):
  - Load k, v; build kT (d,s) tiles via PE transpose. v is augmented with a
    ones column -> v_aug (s, 129) in bf16.
  - Per head h: build qT (d,t) via PE transpose; for each s-tile j compute
    S^T_j = k_j @ q_h^T (s=128 partitions, t>=j*128 free) on PE (bf16,
    f32 PSUM), exp(scale*S^T) on ScalarE straight from PSUM into bf16 SBUF
    (causal diagonal tile masked by an upper-triangular multiply).
  - PV with the e^T blocks as the stationary operand and v_aug streaming:
    out_psum (t=128, 129) accumulates over j; column 128 is the softmax
    denominator. Normalize with a per-partition reciprocal multiply and DMA
    the (t, d) tile to DRAM.
"""

import sys
import functools

import numpy as np

if "/opt/trn_rl_repo" not in sys.path:
    sys.path.insert(0, "/opt/trn_rl_repo")

T = 2048
H_TOTAL = 32
N_CORES = 8
H = H_TOTAL // N_CORES  # 4 q heads per core
D = 128
P = 128
NT = T // P  # 16 token tiles
SCALE = 0.08838834764831845

# column offset of s-tile j's slice inside the per-head packed e^T buffer
_EOFF = [0] * (NT + 1)
for _j in range(NT):
    _EOFF[_j + 1] = _EOFF[_j] + (T - P * _j)
E_COLS = _EOFF[NT]  # 17408


def _n_chunks(n_tiles):
    """Split n_tiles 128-col tiles into matmul chunks of <=4 tiles (<=512 cols)."""
    out = []
    i = 0
    while i < n_tiles:
        c = min(4, n_tiles - i)
        out.append((i, c))
        i += c
    return out


def _build_body(tc, nc, q_d, k_d, v_d, o_d, ctx):
    import concourse.mybir as mybir
    from concourse.masks import make_identity, make_upper_triangular

    f32 = mybir.dt.float32
    bf16 = mybir.dt.bfloat16

    const = ctx.enter_context(tc.tile_pool(name="const", bufs=1))
    qsp = ctx.enter_context(tc.tile_pool(name="qstage", bufs=2))
    qtp = ctx.enter_context(tc.tile_pool(name="qT", bufs=2))
    ep = ctx.enter_context(tc.tile_pool(name="eT", bufs=2))
    outp = ctx.enter_context(tc.tile_pool(name="outt", bufs=4))
    recp = ctx.enter_context(tc.tile_pool(name="rec", bufs=4))

    st_pool = ctx.enter_context(tc.tile_pool(name="st", bufs=1, space="PSUM"))
    sm_pool = ctx.enter_context(tc.tile_pool(name="smp", bufs=3, space="PSUM"))

    identity = const.tile([P, P], f32, tag="ident")
    make_identity(nc, identity)
    utri = const.tile([P, P], bf16, tag="utri")
    make_upper_triangular(nc, utri, val=1.0, diag=True)

    # ---- k, v ----
    k_sb = const.tile([P, NT, D], f32, tag="ksb")
    nc.sync.dma_start(k_sb, k_d.rearrange("(j p) d -> p j d", p=P))
    v_sb = const.tile([P, NT, D], f32, tag="vsb")
    nc.sync.dma_start(v_sb, v_d.rearrange("(j p) d -> p j d", p=P))
    v_aug = const.tile([P, NT, D + 1], bf16, tag="vaug")
    nc.vector.tensor_copy(out=v_aug[:, :, 0:D], in_=v_sb)
    nc.vector.memset(v_aug[:, :, D:D + 1], 1.0)

    kT = const.tile([P, NT, P], bf16, tag="kT")  # [d, j, s]
    for j in range(NT):
        tp = sm_pool.tile([P, P + 1], f32, tag="sm")
        nc.tensor.transpose(tp[:, 0:P], k_sb[:, j, :], identity)
        nc.vector.tensor_copy(out=kT[:, j, :], in_=tp[:, 0:P])

    q_view = q_d.rearrange("(i p) h d -> p i h d", p=P)
    o_view = o_d.rearrange("(i p) h d -> p i h d", p=P)

    def load_qT(h):
        """DMA head h's q and transpose to (d, t) bf16 tiles."""
        q_stage = qsp.tile([P, NT, D], f32, tag="qstage")
        nc.sync.dma_start(q_stage, q_view[:, :, h, :])
        qT = qtp.tile([P, NT, P], bf16, tag="qT")  # [d, i, t]
        for i in range(NT):
            tp = sm_pool.tile([P, P + 1], f32, tag="sm")
            nc.tensor.transpose(tp[:, 0:P], q_stage[:, i, :], identity)
            nc.vector.tensor_copy(out=qT[:, i, :], in_=tp[:, 0:P])
        return qT

    def pv_step(eT, h, i):
        """Accumulate out tile (t-tile i of head h) over s-tiles, normalize, DMA."""
        pv = sm_pool.tile([P, P + 1], f32, tag="sm")
        for j in range(i + 1):
            c0 = _EOFF[j] + (i - j) * P
            nc.tensor.matmul(
                pv,
                lhsT=eT[:, c0:c0 + P],
                rhs=v_aug[:, j, :],
                start=(j == 0),
                stop=(j == i),
            )
        rec = recp.tile([P, 1], f32, tag="rec")
        nc.vector.reciprocal(rec, pv[:, D:D + 1])
        ot = outp.tile([P, D], f32, tag="outt")
        nc.vector.tensor_scalar_mul(ot, pv[:, 0:D], rec)
        nc.sync.dma_start(o_view[:, i, h, :], ot)

    qT = load_qT(0)
    next_qT = None
    for h in range(H):
        if h + 1 < H:
            next_qT = load_qT(h + 1)
        eT = ep.tile([P, E_COLS], bf16, tag="eT")
        for j in range(NT):
            nj = T - P * j
            st = st_pool.tile([P, T], f32, tag="st")
            for (i0, ci) in _n_chunks(NT - j):
                nc.tensor.matmul(
                    st[:, i0 * P:(i0 + ci) * P],
                    lhsT=kT[:, j, :],
                    rhs=qT[:, j + i0:j + i0 + ci, :],
                    start=True,
                    stop=True,
                )
            nc.scalar.activation(
                out=eT[:, _EOFF[j]:_EOFF[j] + nj],
                in_=st[:, 0:nj],
                func=mybir.ActivationFunctionType.Exp,
                scale=SCALE,
            )
            # causal mask on the diagonal tile: keep t_local >= s_local
            nc.vector.tensor_tensor(
                eT[:, _EOFF[j]:_EOFF[j] + P],
                eT[:, _EOFF[j]:_EOFF[j] + P],
                utri,
                mybir.AluOpType.mult,
            )
            if h > 0:
                # PV of the previous head, largest tiles first so the PE work
                # paired with exp(j) shrinks as exp(j) does.
                pv_step(prev_eT, h - 1, NT - 1 - j)
        prev_eT = eT
        qT = next_qT
    for i in range(NT):
        pv_step(prev_eT, H - 1, i)


@functools.lru_cache(maxsize=1)
def _build():
    import concourse.tile as tile
    import concourse.mybir as mybir
    from concourse import bacc
    from contextlib import ExitStack

    f32 = mybir.dt.float32
    nc = bacc.Bacc(
        "TRN2",
        target_bir_lowering=False,
        debug=False,
        num_devices=N_CORES,
    )
    q_d = nc.dram_tensor("q", (T, H, D), f32, kind="ExternalInput").ap()
    k_d = nc.dram_tensor("k", (T, D), f32, kind="ExternalInput").ap()
    v_d = nc.dram_tensor("v", (T, D), f32, kind="ExternalInput").ap()
    o_d = nc.dram_tensor("out", (T, H, D), f32, kind="ExternalOutput").ap()

    with tile.TileContext(nc) as tc:
        with ExitStack() as ctx:
            _build_body(tc, nc, q_d, k_d, v_d, o_d, ctx)
    nc.compile()
    return nc


def _in_maps(q, k, v):
    q = np.asarray(q, dtype=np.float32)
    k = np.asarray(k, dtype=np.float32)
    v = np.asarray(v, dtype=np.float32)
    return [
        {
            "q": np.ascontiguousarray(q[:, H * c:H * c + H, :]),
            "k": np.ascontiguousarray(k[:, c, :]),
            "v": np.ascontiguousarray(v[:, c, :]),
        }
        for c in range(N_CORES)
    ]


def kernel(q, k, v, _trace=False):
    from concourse.bass_utils import run_bass_kernel_spmd

    nc = _build()
    res = run_bass_kernel_spmd(
        nc, _in_maps(q, k, v), core_ids=list(range(N_CORES)), trace=_trace
    )
    out = np.empty((T, H_TOTAL, D), dtype=np.float32)
    for c in range(N_CORES):
        out[:, H * c:H * c + H, :] = res.results[c]["out"].reshape(T, H, D)
    if _trace:
        return out, res
    return out


# revision 5
# speedup vs baseline: 1.0337x; 1.0337x over previous
"""GQA causal prefill attention on 8 TRN2 NeuronCores.

Sharding: head-parallel. Core c computes q heads [4c, 4c+4) against kv head c
(n_rep = 4, so the GQA groups align exactly with the shard; no cross-core
communication).

Per-core algorithm (T=2048 tokens, 4 q heads, head_dim 128):
  - Load k, v; build kT (d,s) tiles via PE transpose. v is augmented with a
    ones column -> v_aug (s, 129) in bf16.
  - Per head h: build qT (d,t) via PE transpose; for each s-tile j compute
    S^T_j = k_j @ q_h^T (s=128 partitions, t>=j*128 free) on PE (bf16,
    f32 PSUM), exp(scale*S^T) on ScalarE straight from PSUM into bf16 SBUF
    (causal diagonal tile masked by an upper-triangular multiply).
  - PV with the e^T blocks as the stationary operand and v_aug streaming:
    out_psum (t=128, 129) accumulates over j; column 128 is the softmax
    denominator. Normalize with a per-partition reciprocal multiply and DMA
    the (t, d) tile to DRAM.
"""

import sys
import functools

import numpy as np

if "/opt/trn_rl_repo" not in sys.path:
    sys.path.insert(0, "/opt/trn_rl_repo")

T = 2048
H_TOTAL = 32
N_CORES = 8
H = H_TOTAL // N_CORES  # 4 q heads per core
D = 128
P = 128
NT = T // P  # 16 token tiles
SCALE = 0.08838834764831845

# column offset of s-tile j's slice inside the per-head packed e^T buffer
_EOFF = [0] * (NT + 1)
for _j in range(NT):
    _EOFF[_j + 1] = _EOFF[_j] + (T - P * _j)
E_COLS = _EOFF[NT]  # 17408


def _n_chunks(n_tiles):
    """Split n_tiles 128-col tiles into matmul chunks of <=4 tiles (<=512 cols)."""
    out = []
    i = 0
    while i < n_tiles:
        c = min(4, n_tiles - i)
        out.append((i, c))
        i += c
    return out


def _build_body(tc, nc, q_d, k_d, v_d, o_d, ctx):
    from collections import deque

    import concourse.mybir as mybir
    from concourse.masks import make_identity, make_upper_triangular

    f32 = mybir.dt.float32
    bf16 = mybir.dt.bfloat16

    const = ctx.enter_context(tc.tile_pool(name="const", bufs=1))
    qsp = ctx.enter_context(tc.tile_pool(name="qstage", bufs=2))
    qbp = ctx.enter_context(tc.tile_pool(name="qbf", bufs=4))
    qtp = ctx.enter_context(tc.tile_pool(name="qT", bufs=4))
    ep = ctx.enter_context(tc.tile_pool(name="eT", bufs=2))
    outp = ctx.enter_context(tc.tile_pool(name="outt", bufs=4))
    recp = ctx.enter_context(tc.tile_pool(name="rec", bufs=4))

    st_pool = ctx.enter_context(tc.tile_pool(name="st", bufs=1, space="PSUM"))
    pv_pool = ctx.enter_context(tc.tile_pool(name="pvp", bufs=2, space="PSUM"))
    tp_pool = ctx.enter_context(tc.tile_pool(name="tpp", bufs=2, space="PSUM"))

    identity = const.tile([P, P], bf16, tag="ident")
    make_identity(nc, identity)
    utri = const.tile([P, P], bf16, tag="utri")
    make_upper_triangular(nc, utri, val=1.0, diag=True)

    # ---- k, v ----
    k_sb = const.tile([P, NT, D], f32, tag="ksb")
    nc.sync.dma_start(k_sb, k_d.rearrange("(j p) d -> p j d", p=P))
    k_bf = const.tile([P, NT, D], bf16, tag="kbf")
    nc.vector.tensor_copy(out=k_bf, in_=k_sb)
    v_sb = const.tile([P, NT, D], f32, tag="vsb")
    nc.sync.dma_start(v_sb, v_d.rearrange("(j p) d -> p j d", p=P))
    v_aug = const.tile([P, NT, D + 1], bf16, tag="vaug")
    nc.vector.tensor_copy(out=v_aug[:, :, 0:D], in_=v_sb)
    nc.vector.memset(v_aug[:, :, D:D + 1], 1.0)

    def transpose_batch(dst, src, b):
        """Transpose 4 (128,128) bf16 tiles src[:, 4b+m, :] into dst[:, 4b+m, :]
        through one 1-bank PSUM tile and a single batched copy."""
        tp = tp_pool.tile([P, 4 * P], bf16, tag="tp")
        for m in range(4):
            nc.tensor.transpose(tp[:, m * P:(m + 1) * P], src[:, 4 * b + m, :], identity)
        nc.vector.tensor_copy(out=dst[:, 4 * b:4 * b + 4, :], in_=tp)

    kT = const.tile([P, NT, P], bf16, tag="kT")  # [d, j, s]
    for b in range(NT // 4):
        transpose_batch(kT, k_bf, b)

    q_view = q_d.rearrange("(i p) h d -> p i h d", p=P)
    o_view = o_d.rearrange("(i p) h d -> p i h d", p=P)

    # Stage + cast q for every head up front (DMA/DVE only); transposes for
    # heads 1..3 are deferred as PE filler work.
    q_bf = []
    for h in range(H):
        q_stage = qsp.tile([P, NT, D], f32, tag="qstage")
        nc.sync.dma_start(q_stage, q_view[:, :, h, :])
        qb = qbp.tile([P, NT, D], bf16, tag="qbf")
        nc.vector.tensor_copy(out=qb, in_=q_stage)
        q_bf.append(qb)

    qT = [
        qtp.tile([P, NT, P], bf16, tag="qT", name=f"qT{h}") for h in range(H)
    ]  # [d, i, t]
    for b in range(NT // 4):
        transpose_batch(qT[0], q_bf[0], b)
    # filler thunks: (head, batch) transposes for heads 1..3
    fillers = deque(
        (h, b) for h in range(1, H) for b in range(NT // 4)
    )

    def emit_fillers_for_head(h):
        while fillers and fillers[0][0] <= h:
            fh, fb = fillers.popleft()
            transpose_batch(qT[fh], q_bf[fh], fb)

    def emit_chain(eT, h, i):
        """PV accumulation for t-tile i of head h: out_psum (t,129); col 128 is
        the softmax denominator. Normalize and DMA out."""
        pv = pv_pool.tile([P, P + 1], f32, tag="pv")
        for j in range(i + 1):
            c0 = _EOFF[j] + (i - j) * P
            nc.tensor.matmul(
                pv,
                lhsT=eT[:, c0:c0 + P],
                rhs=v_aug[:, j, :],
                start=(j == 0),
                stop=(j == i),
            )
        rec = recp.tile([P, 1], f32, tag="rec")
        nc.vector.reciprocal(rec, pv[:, D:D + 1])
        ot = outp.tile([P, D], f32, tag="outt")
        nc.vector.tensor_scalar_mul(ot, pv[:, 0:D], rec)
        nc.sync.dma_start(o_view[:, i, h, :], ot)

    ready = deque()  # (eT, head, i) PV chains not yet emitted

    def pop_ready(budget, force=False):
        while ready:
            e2, h2, i2 = ready[0]
            size = i2 + 1
            if not force and size > budget and budget < 16:
                break
            ready.popleft()
            emit_chain(e2, h2, i2)
            budget -= size
            if budget <= 0 and not force:
                break

    for h in range(H):
        eT = ep.tile([P, E_COLS], bf16, tag="eT")
        for j in range(NT):
            # PE work for the exp(j-1) window FIRST: in-order engine streams
            # mean anything emitted after S^T(j)'s psum-wait would be stuck
            # behind it.
            pop_ready((NT - j) + (6 if h == H - 1 else 2))
            if h == 0 and fillers:
                fh, fb = fillers.popleft()
                transpose_batch(qT[fh], q_bf[fh], fb)
            nj = T - P * j
            st = st_pool.tile([P, T], f32, tag="st")
            for (i0, ci) in _n_chunks(NT - j):
                nc.tensor.matmul(
                    st[:, i0 * P:(i0 + ci) * P],
                    lhsT=kT[:, j, :],
                    rhs=qT[h][:, j + i0:j + i0 + ci, :],
                    start=True,
                    stop=True,
                )
            nc.scalar.activation(
                out=eT[:, _EOFF[j]:_EOFF[j] + nj],
                in_=st[:, 0:nj],
                func=mybir.ActivationFunctionType.Exp,
                scale=SCALE,
            )
            # causal mask on the diagonal tile: keep t_local >= s_local
            nc.vector.tensor_tensor(
                eT[:, _EOFF[j]:_EOFF[j] + P],
                eT[:, _EOFF[j]:_EOFF[j] + P],
                utri,
                mybir.AluOpType.mult,
            )
            ready.append((eT, h, j))
        if h + 1 < H:
            emit_fillers_for_head(h + 1)
        if h >= 1:
            # everything from head h-1 must drain before its eT slot recycles
            while ready and ready[0][1] < h:
                e2, h2, i2 = ready.popleft()
                emit_chain(e2, h2, i2)
    pop_ready(0, force=True)


@functools.lru_cache(maxsize=1)
def _build():
    import concourse.tile as tile
    import concourse.mybir as mybir
    from concourse import bacc
    from contextlib import ExitStack

    f32 = mybir.dt.float32
    nc = bacc.Bacc(
        "TRN2",
        target_bir_lowering=False,
        debug=False,
        num_devices=N_CORES,
    )
    q_d = nc.dram_tensor("q", (T, H, D), f32, kind="ExternalInput").ap()
    k_d = nc.dram_tensor("k", (T, D), f32, kind="ExternalInput").ap()
    v_d = nc.dram_tensor("v", (T, D), f32, kind="ExternalInput").ap()
    o_d = nc.dram_tensor("out", (T, H, D), f32, kind="ExternalOutput").ap()

    with tile.TileContext(nc) as tc:
        with ExitStack() as ctx:
            _build_body(tc, nc, q_d, k_d, v_d, o_d, ctx)
    nc.compile()
    return nc


def _in_maps(q, k, v):
    q = np.asarray(q, dtype=np.float32)
    k = np.asarray(k, dtype=np.float32)
    v = np.asarray(v, dtype=np.float32)
    return [
        {
            "q": np.ascontiguousarray(q[:, H * c:H * c + H, :]),
            "k": np.ascontiguousarray(k[:, c, :]),
            "v": np.ascontiguousarray(v[:, c, :]),
        }
        for c in range(N_CORES)
    ]


def kernel(q, k, v, _trace=False):
    from concourse.bass_utils import run_bass_kernel_spmd

    nc = _build()
    res = run_bass_kernel_spmd(
        nc, _in_maps(q, k, v), core_ids=list(range(N_CORES)), trace=_trace
    )
    out = np.empty((T, H_TOTAL, D), dtype=np.float32)
    for c in range(N_CORES):
        out[:, H * c:H * c + H, :] = res.results[c]["out"].reshape(T, H, D)
    if _trace:
        return out, res
    return out
